# revision 13
# baseline (speedup 1.0000x reference)
"""Differential attention kernel for 8 Trainium2 NeuronCores.

Sharding: core c handles batch b = c//4, query rows [(c%4)*1024, (c%4+1)*1024).
Each core receives x[b]^T (bf16, key-columns rolled so its own query block is
first), computes K^T/Q^T projections W-stationary, V x-stationary, applies RoPE
on the transposed layout, runs both softmax branches with fused denominators
(activation accum_out), combines P = E1 - (lam*d1/d2)*E2, transposes P via the
DMA XBAR, and accumulates P@V on the tensor engine.
"""

import math
from contextlib import ExitStack

import ml_dtypes
import numpy as np

import concourse.bass as bass
import concourse.mybir as mybir
import concourse.tile as tile
from concourse import bacc
from concourse.bass_utils import run_bass_kernel_spmd

B, S, D = 2, 4096, 1024
HD = 64
ROT = 128
NQ = 1024  # query rows per core
N_CORES = 8
DC = D // 128  # contraction chunks for the projections
KQ = 1024  # keys per PSUM score block / projection quarter
NKQ = S // KQ  # 4 quarters
NMAX = 512  # moving-operand cap
FP32 = mybir.dt.float32
BF16 = mybir.dt.bfloat16
AF = mybir.ActivationFunctionType
ALU = mybir.AluOpType

_prog_cache = {}


def _build_kernel(ctx: ExitStack, tc, xT, w_sb_aps, cosT, sinT, lamn, out):
    nc = tc.nc

    const = ctx.enter_context(tc.tile_pool(name="const", bufs=1))

    w_sb = {}
    for name, ap in w_sb_aps.items():
        t = const.tile([128, DC, ROT], BF16, tag=name)
        nc.sync.dma_start(t[:], ap.rearrange("(a p) r -> p a r", p=128))
        w_sb[name] = t

    cos_sb = const.tile([128, S], BF16, tag="cos")
    nc.sync.dma_start(cos_sb[:], cosT)
    sin_sb = const.tile([128, S], BF16, tag="sin")
    nc.sync.dma_start(sin_sb[:], sinT)
    lam_sb = const.tile([128, 1], FP32, tag="lam")
    nc.sync.dma_start(lam_sb[:], lamn)

    # per-quarter K^T tiles so attention on quarter h only waits for its rope
    kT = []
    for h in range(NKQ):
        kTh = const.tile([128, KQ], BF16, tag=f"kT{h}")
        kT.append(kTh)
    qT = const.tile([128, NQ], BF16, tag="qT")
    v_sb = const.tile([128, S // 128, ROT], BF16, tag="v")

    # ---- Phase A: projections + rope ----
    with (
        tc.tile_pool(name="xt", bufs=1) as xt_pool,
        tc.tile_pool(name="psA", bufs=2, space="PSUM") as psA,
        tc.tile_pool(name="psV", bufs=4, space="PSUM") as psV,
        tc.tile_pool(name="ropetmp", bufs=2) as ropetmp,
    ):
        # quarter-major loads: one DMA brings all D-chunks for KQ key columns
        xTr = xT.rearrange("(a p) s -> p a s", p=128)
        xt_q = []
        for h in range(NKQ):
            t = xt_pool.tile([128, DC, KQ], BF16, tag=f"xt{h}")
            nc.sync.dma_start(t[:], xTr[:, :, h * KQ : (h + 1) * KQ])
            xt_q.append(t)

        def project_rope(dst, width, s_off, w_tile):
            xq = xt_q[s_off // KQ]
            ps = psA.tile([128, width], FP32, tag="kq")
            for nn in range(width // NMAX):
                for dc in range(DC):
                    nc.tensor.matmul(
                        ps[:, nn * NMAX : (nn + 1) * NMAX],
                        lhsT=w_tile[:, dc, :],
                        rhs=xq[:, dc, nn * NMAX : (nn + 1) * NMAX],
                        start=(dc == 0),
                        stop=(dc == DC - 1),
                    )
            csl = slice(s_off, s_off + width)
            t1 = ropetmp.tile([128, width], FP32, tag="t1")
            t2 = ropetmp.tile([128, width], FP32, tag="t2")
            nc.vector.tensor_mul(t1[:], ps[:], cos_sb[:, csl])
            nc.vector.tensor_mul(t2[0:64, :], ps[64:128, :], sin_sb[0:64, csl])
            nc.vector.tensor_mul(t2[64:128, :], ps[0:64, :], sin_sb[64:128, csl])
            nc.vector.tensor_add(dst, t1[:], t2[:])

        project_rope(qT[:, :], NQ, 0, w_sb["wq"])
        for h in range(NKQ):
            project_rope(kT[h][:, :], KQ, h * KQ, w_sb["wk"])

        for st in range(S // 128):
            xq = xt_q[st * 128 // KQ]
            so = (st * 128) % KQ
            psv = psV.tile([128, ROT], FP32, tag="v")
            for dc in range(DC):
                nc.tensor.matmul(
                    psv[:],
                    lhsT=xq[:, dc, so : so + 128],
                    rhs=w_sb["wv"][:, dc, :],
                    start=(dc == 0),
                    stop=(dc == DC - 1),
                )
            nc.vector.tensor_copy(v_sb[:, st, :], psv[:])

    # ---- Phase B: attention ----
    work = ctx.enter_context(tc.tile_pool(name="work", bufs=2))
    dstat = ctx.enter_context(tc.tile_pool(name="dstat", bufs=2))
    with tc.tile_pool(name="psB", bufs=1, space="PSUM") as psB:
        for qt in range(NQ // 128):
            qsl = slice(qt * 128, (qt + 1) * 128)
            e1 = work.tile([128, S], BF16, tag="e1")
            e2 = work.tile([128, S], BF16, tag="e2")
            d1p = dstat.tile([128, NKQ], FP32, tag="d1p")
            d2p = dstat.tile([128, NKQ], FP32, tag="d2p")
            for h in range(NKQ):
                for br in range(2):
                    psc = psB.tile([128, KQ], FP32, tag=f"s{(h * 2 + br) % 3}")
                    lo, hi = (0, 64) if br == 0 else (64, 128)
                    for nn in range(KQ // NMAX):
                        nc.tensor.matmul(
                            psc[:, nn * NMAX : (nn + 1) * NMAX],
                            lhsT=qT[lo:hi, qsl],
                            rhs=kT[h][lo:hi, nn * NMAX : (nn + 1) * NMAX],
                            start=True,
                            stop=True,
                            tile_position=(br * 64, 0),
                        )
                    e_t, d_t = (e1, d1p) if br == 0 else (e2, d2p)
                    nc.scalar.activation(
                        e_t[:, h * KQ : (h + 1) * KQ], psc[:], AF.Exp,
                        scale=HD**-0.5, accum_out=d_t[:, h : h + 1],
                    )

            d1 = dstat.tile([128, 1], FP32, tag="d1")
            d2 = dstat.tile([128, 1], FP32, tag="d2")
            nc.vector.reduce_sum(d1[:], d1p[:], axis=mybir.AxisListType.X)
            nc.vector.reduce_sum(d2[:], d2p[:], axis=mybir.AxisListType.X)
            r1 = dstat.tile([128, 1], FP32, tag="r1")
            r2 = dstat.tile([128, 1], FP32, tag="r2")
            nc.vector.reciprocal(r1[:], d1[:])
            nc.vector.reciprocal(r2[:], d2[:])
            c2n = dstat.tile([128, 1], FP32, tag="c2n")
            nc.vector.tensor_mul(c2n[:], d1[:], r2[:])
            nc.vector.tensor_mul(c2n[:], c2n[:], lam_sb[:])

            p_t = work.tile([128, S], BF16, tag="p")
            for h in range(2):
                sl = slice(h * (S // 2), (h + 1) * (S // 2))
                nc.vector.scalar_tensor_tensor(
                    out=p_t[:, sl], in0=e2[:, sl], scalar=c2n[:], in1=e1[:, sl],
                    op0=ALU.mult, op1=ALU.add,
                )

            # One XBAR transpose per row-block: pT[p, t, q] = P[q, 128t+p],
            # i.e. pT[:, t, :] is P^T rows [128t, 128t+128) — the PV lhsT tiles.
            pT = work.tile([128, S // 128, 128], BF16, tag="pT")
            nc.sync.dma_start(pT[:], p_t[:], transpose=True)

            pso = psB.tile([128, ROT], FP32, tag="o")
            for kc in range(S // 128):
                nc.tensor.matmul(
                    pso[:], lhsT=pT[:, kc, :], rhs=v_sb[:, kc, :],
                    start=(kc == 0), stop=(kc == S // 128 - 1),
                )
            o_t = work.tile([128, ROT], FP32, tag="o")
            nc.scalar.activation(o_t[:], pso[:], AF.Copy, scale=r1[:])
            nc.sync.dma_start(out[qt * 128 : (qt + 1) * 128, :], o_t[:])


def _get_program(repeat=1):
    if repeat in _prog_cache:
        return _prog_cache[repeat]
    nc = bacc.Bacc("TRN2", target_bir_lowering=False, debug=False, num_devices=N_CORES)
    xT = nc.dram_tensor("xT", [D, S], BF16, kind="ExternalInput").ap()
    wq = nc.dram_tensor("wq", [D, ROT], BF16, kind="ExternalInput").ap()
    wk = nc.dram_tensor("wk", [D, ROT], BF16, kind="ExternalInput").ap()
    wv = nc.dram_tensor("wv", [D, ROT], BF16, kind="ExternalInput").ap()
    cosT = nc.dram_tensor("cosT", [ROT, S], BF16, kind="ExternalInput").ap()
    sinT = nc.dram_tensor("sinT", [ROT, S], BF16, kind="ExternalInput").ap()
    lamn = nc.dram_tensor("lamn", [128, 1], FP32, kind="ExternalInput").ap()
    out = nc.dram_tensor("out", [NQ, ROT], FP32, kind="ExternalOutput").ap()

    with tile.TileContext(nc) as tc:
        for _ in range(repeat):
            with ExitStack() as ctx:
                _build_kernel(
                    ctx, tc, xT, {"wq": wq, "wk": wk, "wv": wv}, cosT, sinT, lamn, out
                )
    nc.compile()
    _prog_cache[repeat] = nc
    return nc


def make_in_maps(x, Wq, Wk, Wv, lambda_q1, lambda_q2, lambda_k1, lambda_k2):
    x = np.asarray(x, dtype=np.float32)
    Wq, Wk, Wv = (np.asarray(w, dtype=np.float32) for w in (Wq, Wk, Wv))
    lq1, lq2, lk1, lk2 = (
        np.asarray(v, dtype=np.float32)
        for v in (lambda_q1, lambda_q2, lambda_k1, lambda_k2)
    )

    lam_init = 0.8 - 0.6 * math.exp(-0.3 * 1)
    lam = float(
        np.exp(np.sum(lq1 * lk1)) - np.exp(np.sum(lq2 * lk2)) + lam_init
    )

    inv = 1.0 / (10000.0 ** (np.arange(0, ROT, 2, dtype=np.float32) / ROT))
    freqs = np.arange(S, dtype=np.float32)[:, None] * inv[None, :]  # [S, 64]
    cosh = np.cos(freqs)
    sinh = np.sin(freqs)
    cosT_full = np.concatenate([cosh, cosh], axis=1).T  # [128, S]
    sinT_full = np.concatenate([-sinh, sinh], axis=1).T

    bf = ml_dtypes.bfloat16
    wq_b, wk_b, wv_b = (np.ascontiguousarray(w, dtype=bf) for w in (Wq, Wk, Wv))
    lam_arr = np.full((128, 1), -lam, dtype=np.float32)

    in_maps = []
    for c in range(N_CORES):
        b, qoff = c // 4, (c % 4) * NQ
        xTr = np.roll(x[b].T, -qoff, axis=1)
        in_maps.append(
            {
                "xT": np.ascontiguousarray(xTr, dtype=bf),
                "wq": wq_b, "wk": wk_b, "wv": wv_b,
                "cosT": np.ascontiguousarray(np.roll(cosT_full, -qoff, axis=1), dtype=bf),
                "sinT": np.ascontiguousarray(np.roll(sinT_full, -qoff, axis=1), dtype=bf),
                "lamn": lam_arr,
            }
        )
    return in_maps


def assemble_out(results):
    outs = [np.asarray(results[c]["out"], dtype=np.float32) for c in range(N_CORES)]
    return np.stack(
        [np.concatenate(outs[0:4], axis=0), np.concatenate(outs[4:8], axis=0)]
    )


def kernel(x, Wq, Wk, Wv, lambda_q1, lambda_q2, lambda_k1, lambda_k2):
    in_maps = make_in_maps(x, Wq, Wk, Wv, lambda_q1, lambda_q2, lambda_k1, lambda_k2)
    nc = _get_program()
    res = run_bass_kernel_spmd(nc, in_maps, list(range(N_CORES)))
    return assemble_out(res.results)


# revision 34
# speedup vs baseline: 16861.2923x; 16861.2923x over previous
"""Differential attention kernel for 8 Trainium2 NeuronCores.

Sharding: core c handles batch b = c//4, query rows [(c%4)*1024, (c%4+1)*1024).
Each core receives x[b]^T (bf16, key-columns rolled so its own query block is
first), computes K^T/Q^T projections W-stationary, V x-stationary, applies RoPE
on the transposed layout, runs both softmax branches with fused denominators
(activation accum_out), combines P = E1 - (lam*d1/d2)*E2, transposes P via the
DMA XBAR, and accumulates P@V on the tensor engine.
"""

import math
from contextlib import ExitStack

import ml_dtypes
import numpy as np

import concourse.bass as bass
import concourse.mybir as mybir
import concourse.tile as tile
from concourse import bacc
from concourse.bass_utils import run_bass_kernel_spmd
from concourse.tile import add_dep_helper

B, S, D = 2, 4096, 1024
HD = 64
ROT = 128
NQ = 1024  # query rows per core
N_CORES = 8
DC = D // 128  # contraction chunks for the projections
KQ = 1024  # keys per PSUM score block / projection quarter
NKQ = S // KQ  # 4 quarters
NMAX = 512  # moving-operand cap
FP32 = mybir.dt.float32
BF16 = mybir.dt.bfloat16
AF = mybir.ActivationFunctionType
ALU = mybir.AluOpType

_prog_cache = {}
CHAIN_DMA = False


def _build_kernel(ctx: ExitStack, tc, xT, w_sb_aps, cosT, sinT, lamn, out):
    nc = tc.nc

    const = ctx.enter_context(tc.tile_pool(name="const", bufs=1))

    # chain input DMAs so early quarters finish early instead of all
    # transfers sharing HBM bandwidth and completing together
    dma_chain = [None]

    def chained_dma(dst, src):
        inst = nc.sync.dma_start(dst, src).ins
        if CHAIN_DMA and dma_chain[0] is not None:
            add_dep_helper(dma_chain[0], inst, reason="serialize input DMA arrivals")
        dma_chain[0] = inst

    w_sb = {}
    for name, ap in w_sb_aps.items():
        t = const.tile([128, DC, ROT], BF16, tag=name)
        chained_dma(t[:], ap.rearrange("(a p) r -> p a r", p=128))
        w_sb[name] = t

    lam_sb = const.tile([128, 1], FP32, tag="lam")
    chained_dma(lam_sb[:], lamn)
    cos_sb = const.tile([128, S], BF16, tag="cos")
    sin_sb = const.tile([128, S], BF16, tag="sin")

    # per-quarter K^T tiles so attention on quarter h only waits for its rope
    kT = []
    for h in range(NKQ):
        kTh = const.tile([128, KQ], BF16, tag=f"kT{h}")
        kT.append(kTh)
    qT = const.tile([128, NQ], BF16, tag="qT")
    v_sb = const.tile([128, S // 128, ROT], BF16, tag="v")

    # ---- Phase A: projections + rope ----
    with (
        tc.tile_pool(name="xt", bufs=1) as xt_pool,
        tc.tile_pool(name="psA", bufs=2, space="PSUM") as psA,
        tc.tile_pool(name="psV", bufs=4, space="PSUM") as psV,
        tc.tile_pool(name="ropetmp", bufs=2) as ropetmp,
    ):
        # quarter-major loads: one DMA brings all D-chunks for KQ key columns
        xTr = xT.rearrange("(a p) s -> p a s", p=128)
        xt_q = []
        for h in range(NKQ):
            t = xt_pool.tile([128, DC, KQ], BF16, tag=f"xt{h}")
            ksl = slice(h * KQ, (h + 1) * KQ)
            chained_dma(t[:], xTr[:, :, ksl])
            chained_dma(cos_sb[:, ksl], cosT[:, ksl])
            chained_dma(sin_sb[:, ksl], sinT[:, ksl])
            xt_q.append(t)

        def project_rope(dst, width, s_off, w_tile):
            xq = xt_q[s_off // KQ]
            ps = psA.tile([128, width], FP32, tag="kq")
            for nn in range(width // NMAX):
                for dc in range(DC):
                    nc.tensor.matmul(
                        ps[:, nn * NMAX : (nn + 1) * NMAX],
                        lhsT=w_tile[:, dc, :],
                        rhs=xq[:, dc, nn * NMAX : (nn + 1) * NMAX],
                        start=(dc == 0),
                        stop=(dc == DC - 1),
                    )
            csl = slice(s_off, s_off + width)
            t1 = ropetmp.tile([128, width], FP32, tag="t1")
            t2 = ropetmp.tile([128, width], FP32, tag="t2")
            nc.vector.tensor_mul(t1[:], ps[:], cos_sb[:, csl])
            nc.vector.tensor_mul(t2[0:64, :], ps[64:128, :], sin_sb[0:64, csl])
            nc.vector.tensor_mul(t2[64:128, :], ps[0:64, :], sin_sb[64:128, csl])
            nc.gpsimd.tensor_add(dst, t1[:], t2[:])

        project_rope(qT[:, :], NQ, 0, w_sb["wq"])
        for h in range(NKQ):
            project_rope(kT[h][:, :], KQ, h * KQ, w_sb["wk"])

        for st in range(S // 128):
            xq = xt_q[st * 128 // KQ]
            so = (st * 128) % KQ
            psv = psV.tile([128, ROT], FP32, tag="v")
            for dc in range(DC):
                nc.tensor.matmul(
                    psv[:],
                    lhsT=xq[:, dc, so : so + 128],
                    rhs=w_sb["wv"][:, dc, :],
                    start=(dc == 0),
                    stop=(dc == DC - 1),
                )
            nc.vector.tensor_copy(v_sb[:, st, :], psv[:])

    # ---- Phase B: attention ----
    work = ctx.enter_context(tc.tile_pool(name="work", bufs=2))
    dstat = ctx.enter_context(tc.tile_pool(name="dstat", bufs=2))
    with tc.tile_pool(name="psB", bufs=1, space="PSUM") as psB:
        for qt in range(NQ // 128):
            qsl = slice(qt * 128, (qt + 1) * 128)
            e1 = work.tile([128, S], BF16, tag="e1")
            e2 = work.tile([128, S], BF16, tag="e2")
            d1p = dstat.tile([128, NKQ], FP32, tag="d1p")
            d2p = dstat.tile([128, NKQ], FP32, tag="d2p")
            for h in range(NKQ):
                for br in range(2):
                    psc = psB.tile([128, KQ], FP32, tag=f"s{(h * 2 + br) % 3}")
                    lo, hi = (0, 64) if br == 0 else (64, 128)
                    for nn in range(KQ // NMAX):
                        nc.tensor.matmul(
                            psc[:, nn * NMAX : (nn + 1) * NMAX],
                            lhsT=qT[lo:hi, qsl],
                            rhs=kT[h][lo:hi, nn * NMAX : (nn + 1) * NMAX],
                            start=True,
                            stop=True,
                            tile_position=(br * 64, 0),
                        )
                    e_t, d_t = (e1, d1p) if br == 0 else (e2, d2p)
                    nc.scalar.activation(
                        e_t[:, h * KQ : (h + 1) * KQ], psc[:], AF.Exp,
                        scale=HD**-0.5, accum_out=d_t[:, h : h + 1],
                    )

            d1 = dstat.tile([128, 1], FP32, tag="d1")
            d2 = dstat.tile([128, 1], FP32, tag="d2")
            nc.vector.reduce_sum(d1[:], d1p[:], axis=mybir.AxisListType.X)
            nc.vector.reduce_sum(d2[:], d2p[:], axis=mybir.AxisListType.X)
            r1 = dstat.tile([128, 1], FP32, tag="r1")
            r2 = dstat.tile([128, 1], FP32, tag="r2")
            nc.vector.reciprocal(r1[:], d1[:])
            nc.vector.reciprocal(r2[:], d2[:])
            c2n = dstat.tile([128, 1], FP32, tag="c2n")
            nc.vector.tensor_mul(c2n[:], d1[:], r2[:])
            nc.vector.tensor_mul(c2n[:], c2n[:], lam_sb[:])

            p_t = work.tile([128, S], BF16, tag="p")
            for h in range(2):
                sl = slice(h * (S // 2), (h + 1) * (S // 2))
                nc.vector.scalar_tensor_tensor(
                    out=p_t[:, sl], in0=e2[:, sl], scalar=c2n[:], in1=e1[:, sl],
                    op0=ALU.mult, op1=ALU.add,
                )

            # XBAR transpose halves: pT[p, t, q] = P[q, 128t+p],
            # i.e. pT[:, t, :] is P^T rows [128t, 128t+128) — the PV lhsT tiles.
            pT = work.tile([128, S // 128, 128], BF16, tag="pT")
            for th in range(2):
                nc.sync.dma_start(
                    pT[:, th * 16 : (th + 1) * 16, :],
                    p_t[:, th * (S // 2) : (th + 1) * (S // 2)],
                    transpose=True,
                )

            pso = psB.tile([128, ROT], FP32, tag=f"o{qt % 2}")
            for kc in range(S // 128):
                nc.tensor.matmul(
                    pso[:], lhsT=pT[:, kc, :], rhs=v_sb[:, kc, :],
                    start=(kc == 0), stop=(kc == S // 128 - 1),
                )
            o_t = work.tile([128, ROT], FP32, tag="o")
            nc.vector.tensor_scalar_mul(o_t[:], pso[:], r1[:])
            nc.sync.dma_start(out[qt * 128 : (qt + 1) * 128, :], o_t[:])


def _get_program(repeat=1):
    if repeat in _prog_cache:
        return _prog_cache[repeat]
    nc = bacc.Bacc("TRN2", target_bir_lowering=False, debug=False, num_devices=N_CORES)
    xT = nc.dram_tensor("xT", [D, S], BF16, kind="ExternalInput").ap()
    wq = nc.dram_tensor("wq", [D, ROT], BF16, kind="ExternalInput").ap()
    wk = nc.dram_tensor("wk", [D, ROT], BF16, kind="ExternalInput").ap()
    wv = nc.dram_tensor("wv", [D, ROT], BF16, kind="ExternalInput").ap()
    cosT = nc.dram_tensor("cosT", [ROT, S], BF16, kind="ExternalInput").ap()
    sinT = nc.dram_tensor("sinT", [ROT, S], BF16, kind="ExternalInput").ap()
    lamn = nc.dram_tensor("lamn", [128, 1], FP32, kind="ExternalInput").ap()
    out = nc.dram_tensor("out", [NQ, ROT], FP32, kind="ExternalOutput").ap()

    with tile.TileContext(nc) as tc:
        for rep in range(repeat):
            if rep > 0:
                # isolate repeated bodies so timing slopes measure single-run latency
                tc.strict_bb_all_engine_barrier()
            with ExitStack() as ctx:
                _build_kernel(
                    ctx, tc, xT, {"wq": wq, "wk": wk, "wv": wv}, cosT, sinT, lamn, out
                )
    nc.compile()
    _prog_cache[repeat] = nc
    return nc


def make_in_maps(x, Wq, Wk, Wv, lambda_q1, lambda_q2, lambda_k1, lambda_k2):
    x = np.asarray(x, dtype=np.float32)
    Wq, Wk, Wv = (np.asarray(w, dtype=np.float32) for w in (Wq, Wk, Wv))
    lq1, lq2, lk1, lk2 = (
        np.asarray(v, dtype=np.float32)
        for v in (lambda_q1, lambda_q2, lambda_k1, lambda_k2)
    )

    lam_init = 0.8 - 0.6 * math.exp(-0.3 * 1)
    lam = float(
        np.exp(np.sum(lq1 * lk1)) - np.exp(np.sum(lq2 * lk2)) + lam_init
    )

    inv = 1.0 / (10000.0 ** (np.arange(0, ROT, 2, dtype=np.float32) / ROT))
    freqs = np.arange(S, dtype=np.float32)[:, None] * inv[None, :]  # [S, 64]
    cosh = np.cos(freqs)
    sinh = np.sin(freqs)
    cosT_full = np.concatenate([cosh, cosh], axis=1).T  # [128, S]
    sinT_full = np.concatenate([-sinh, sinh], axis=1).T

    bf = ml_dtypes.bfloat16
    wq_b, wk_b, wv_b = (np.ascontiguousarray(w, dtype=bf) for w in (Wq, Wk, Wv))
    lam_arr = np.full((128, 1), -lam, dtype=np.float32)

    in_maps = []
    for c in range(N_CORES):
        b, qoff = c // 4, (c % 4) * NQ
        xTr = np.roll(x[b].T, -qoff, axis=1)
        in_maps.append(
            {
                "xT": np.ascontiguousarray(xTr, dtype=bf),
                "wq": wq_b, "wk": wk_b, "wv": wv_b,
                "cosT": np.ascontiguousarray(np.roll(cosT_full, -qoff, axis=1), dtype=bf),
                "sinT": np.ascontiguousarray(np.roll(sinT_full, -qoff, axis=1), dtype=bf),
                "lamn": lam_arr,
            }
        )
    return in_maps


def assemble_out(results):
    outs = [np.asarray(results[c]["out"], dtype=np.float32) for c in range(N_CORES)]
    return np.stack(
        [np.concatenate(outs[0:4], axis=0), np.concatenate(outs[4:8], axis=0)]
    )


def kernel(x, Wq, Wk, Wv, lambda_q1, lambda_q2, lambda_k1, lambda_k2):
    in_maps = make_in_maps(x, Wq, Wk, Wv, lambda_q1, lambda_q2, lambda_k1, lambda_k2)
    nc = _get_program()
    res = run_bass_kernel_spmd(nc, in_maps, list(range(N_CORES)))
    return assemble_out(res.results)
